# revision 4
# baseline (speedup 1.0000x reference)
"""Trainium2 Bass kernel for CombinedLoss_dynamic.

loss = mean((output-target)^2)
     + mean(((output-output_past)/dt - ALPHA*lap3d(input) - SRC*(input>THR))^2)

Sharding: data-parallel over batch. 16 batches / 8 cores = 2 per core.
Per-core layout packs (batch, D) on the 128 SBUF partitions (p = b*64 + d),
with (H, W) on the free axis, chunked over H (16 rows + 1-row halo).

The whole residual is assembled in PSUM by the tensor engine:
  - D-taps + center: block-tridiagonal matmul (partition dim = b*64+d)
  - H-taps:  identity matmuls against H-shifted SBUF views
  - W-taps:  identity matmuls with W-shifted views (strided PSUM out)
  - temporal term: diag(1/dt_b) matmul of (output - output_past)
  - source term: diag(-SRC) matmuls of the threshold mask, with SRC split
    into bf16 hi+lo parts across two matmuls for fp32-level accuracy
All matmul operands are bf16 (weights {1,-6,...} are bf16-exact; operand
rounding noise is ~1e-5 relative on the final loss). The scalar engine
square-accumulates straight out of PSUM; the host only sums the per-core
per-partition partials and divides by N.
"""

import sys

sys.path.insert(0, "/opt/trn_rl_repo")

from contextlib import ExitStack

import ml_dtypes
import numpy as np

import concourse.bacc as bacc
import concourse.tile as tile
from concourse import mybir
from concourse.bass_utils import run_bass_kernel_spmd

# constants matching the reference
ALPHA = np.float32(0.0257)
NORM = 27353.34765625
SRC_INTENSITY = np.float32(100000.0 / NORM)
FIRE_THRESHOLD = np.float32((1000.0 - 20.0) / NORM)

B, D, H, W = 16, 64, 128, 128
N_CORES = 8
BPC = B // N_CORES  # batches per core = 2
P = BPC * D  # 128 partitions = (b_local, d)
HD = 16  # H rows per chunk
NCH = H // HD  # 8 chunks
NBANK = HD * W * 4 // 2048  # 4 PSUM banks per chunk
N_TOTAL = B * D * H * W

F32 = mybir.dt.float32
BF16 = mybir.dt.bfloat16

_NC = None


def _build_nc():
    nc = bacc.Bacc(
        "TRN2", target_bir_lowering=False, debug=False, num_devices=N_CORES
    )
    x = nc.dram_tensor("x", [P, H, W], F32, kind="ExternalInput").ap()
    o = nc.dram_tensor("o", [P, H, W], F32, kind="ExternalInput").ap()
    op = nc.dram_tensor("op", [P, H, W], F32, kind="ExternalInput").ap()
    tg = nc.dram_tensor("tg", [P, H, W], F32, kind="ExternalInput").ap()
    wmd = nc.dram_tensor("wmd", [P, P], BF16, kind="ExternalInput").ap()
    wid = nc.dram_tensor("wid", [P, P], BF16, kind="ExternalInput").ap()
    wdt = nc.dram_tensor("wdt", [P, P], BF16, kind="ExternalInput").ap()
    wsh = nc.dram_tensor("wsh", [P, P], BF16, kind="ExternalInput").ap()
    wsl = nc.dram_tensor("wsl", [P, P], BF16, kind="ExternalInput").ap()
    accp = nc.dram_tensor("accp", [P, NCH], F32, kind="ExternalOutput").ap()
    accm = nc.dram_tensor("accm", [P, NCH], F32, kind="ExternalOutput").ap()

    Sq = mybir.ActivationFunctionType.Square
    Copy = mybir.ActivationFunctionType.Copy
    sub = mybir.AluOpType.subtract
    is_gt = mybir.AluOpType.is_gt

    with tile.TileContext(nc) as tc, ExitStack() as ctx:
        wpool = ctx.enter_context(tc.tile_pool(name="w", bufs=1))
        apool = ctx.enter_context(tc.tile_pool(name="acc", bufs=1))
        xpool = ctx.enter_context(tc.tile_pool(name="xt", bufs=3))
        xspool = ctx.enter_context(tc.tile_pool(name="xs", bufs=2))
        iopool = ctx.enter_context(tc.tile_pool(name="io", bufs=3))
        bfpool = ctx.enter_context(tc.tile_pool(name="bf", bufs=2))
        mpool = ctx.enter_context(tc.tile_pool(name="m", bufs=2))
        dpool = ctx.enter_context(tc.tile_pool(name="dump", bufs=4))
        pspool = ctx.enter_context(tc.tile_pool(name="ps", bufs=2, space="PSUM"))

        t_md = wpool.tile([P, P], BF16, tag="wmd")
        nc.sync.dma_start(t_md[:], wmd[:])
        t_id = wpool.tile([P, P], BF16, tag="wid")
        nc.sync.dma_start(t_id[:], wid[:])
        t_dt = wpool.tile([P, P], BF16, tag="wdt")
        nc.sync.dma_start(t_dt[:], wdt[:])
        t_sh = wpool.tile([P, P], BF16, tag="wsh")
        nc.sync.dma_start(t_sh[:], wsh[:])
        t_sl = wpool.tile([P, P], BF16, tag="wsl")
        nc.sync.dma_start(t_sl[:], wsl[:])

        t_accp = apool.tile([P, NCH], F32, tag="accp")
        t_accm = apool.tile([P, NCH], F32, tag="accm")

        for ci in range(NCH):
            h0 = ci * HD
            # input tile with 1-row halo on both sides (tile row r = h0-1+r)
            xt = xpool.tile([P, HD + 2, W], F32, tag="xt")
            if ci == 0:
                nc.vector.memset(xt[:, 0:1, :], 0.0)
                nc.sync.dma_start(xt[:, 1 : HD + 2, :], x[:, 0 : HD + 1, :])
            elif ci == NCH - 1:
                nc.vector.memset(xt[:, HD + 1 : HD + 2, :], 0.0)
                nc.sync.dma_start(xt[:, 0 : HD + 1, :], x[:, h0 - 1 : H, :])
            else:
                nc.sync.dma_start(xt[:], x[:, h0 - 1 : h0 + HD + 1, :])

            # xs = bf16(-ALPHA * x), halo rows included
            xs = xspool.tile([P, HD + 2, W], BF16, tag="xs")
            nc.scalar.activation(xs[:], xt[:], Copy, bias=0.0, scale=float(-ALPHA))

            to = iopool.tile([P, HD, W], F32, tag="o")
            nc.sync.dma_start(to[:], o[:, h0 : h0 + HD, :])
            top = iopool.tile([P, HD, W], F32, tag="op")
            nc.sync.dma_start(top[:], op[:, h0 : h0 + HD, :])
            ttg = iopool.tile([P, HD, W], F32, tag="tg")
            nc.sync.dma_start(ttg[:], tg[:, h0 : h0 + HD, :])

            d1 = bfpool.tile([P, HD, W], BF16, tag="d1")
            nc.vector.tensor_tensor(d1[:], to[:], top[:], sub)
            s = bfpool.tile([P, HD, W], BF16, tag="s")
            nc.gpsimd.tensor_scalar(s[:], xt[:, 1 : HD + 1, :], float(FIRE_THRESHOLD), None, is_gt)
            m = mpool.tile([P, HD, W], F32, tag="m")
            nc.vector.tensor_tensor(m[:], to[:], ttg[:], sub)

            # assemble the residual in one 4-bank PSUM tile (bank = 4 H-rows)
            RB = HD // NBANK  # 4 rows per bank
            pt = pspool.tile([P, HD, W], F32, tag="pt")
            for k in range(NBANK):  # D-taps + center
                r0 = 1 + RB * k
                nc.tensor.matmul(pt[:, RB * k : RB * k + RB, :], t_md[:], xs[:, r0 : r0 + RB, :], start=True, stop=False)
            for k in range(NBANK):  # H-taps
                r0 = 1 + RB * k
                nc.tensor.matmul(pt[:, RB * k : RB * k + RB, :], t_id[:], xs[:, r0 - 1 : r0 + RB - 1, :], start=False, stop=False)
                nc.tensor.matmul(pt[:, RB * k : RB * k + RB, :], t_id[:], xs[:, r0 + 1 : r0 + RB + 1, :], start=False, stop=False)
            for k in range(NBANK):  # + (o - op)/dt
                nc.tensor.matmul(pt[:, RB * k : RB * k + RB, :], t_dt[:], d1[:, RB * k : RB * k + RB, :], start=False, stop=False)
            for k in range(NBANK):  # - SRC_hi * mask
                nc.tensor.matmul(pt[:, RB * k : RB * k + RB, :], t_sh[:], s[:, RB * k : RB * k + RB, :], start=False, stop=False)
            for k in range(NBANK):  # - SRC_lo * mask
                nc.tensor.matmul(pt[:, RB * k : RB * k + RB, :], t_sl[:], s[:, RB * k : RB * k + RB, :], start=False, stop=True)

            # W-taps: in-place shifted adds on PSUM
            add = mybir.AluOpType.add
            nc.vector.tensor_tensor(pt[:, :, 1:W], pt[:, :, 1:W], xs[:, 1 : HD + 1, 0 : W - 1], add)
            nc.vector.tensor_tensor(pt[:, :, 0 : W - 1], pt[:, :, 0 : W - 1], xs[:, 1 : HD + 1, 1:W], add)

            dk = dpool.tile([P, HD, W], F32, tag="dk")
            nc.scalar.activation(dk[:], pt[:], Sq, accum_out=t_accp[:, ci : ci + 1])
            dm = dpool.tile([P, HD, W], F32, tag="dm")
            nc.scalar.activation(dm[:], m[:], Sq, accum_out=t_accm[:, ci : ci + 1])

        nc.sync.dma_start(accp[:], t_accp[:])
        nc.sync.dma_start(accm[:], t_accm[:])

    nc.compile()
    return nc


def get_nc():
    global _NC
    if _NC is None:
        _NC = _build_nc()
    return _NC


def _bf16(a):
    return np.asarray(a, dtype=ml_dtypes.bfloat16)


def build_in_maps(input, output, output_past, t, t_past, target):
    # stencil matrix over partitions p = b_local*64 + d: block tridiag(1,-6,1)
    md = np.zeros((P, P), np.float32)
    for bl in range(BPC):
        for d in range(D):
            i = bl * D + d
            md[i, i] = -6.0
            if d > 0:
                md[i, i - 1] = 1.0
            if d < D - 1:
                md[i, i + 1] = 1.0
    ident = np.eye(P, dtype=np.float32)

    # -SRC split into bf16 hi+lo so the pair sums to fp32(SRC) almost exactly
    src_hi = np.float32(ml_dtypes.bfloat16(SRC_INTENSITY))
    src_lo = np.float32(SRC_INTENSITY) - src_hi
    wsh = ident * (-src_hi)
    wsl = ident * (-src_lo)

    dt = (np.asarray(t, np.float32) - np.asarray(t_past, np.float32))[:, 0]  # [B]

    in_maps = []
    for c in range(N_CORES):
        inv_dt_p = np.repeat(1.0 / dt[c * BPC : (c + 1) * BPC], D).astype(np.float32)
        in_maps.append(
            {
                "x": np.ascontiguousarray(input[c * BPC : (c + 1) * BPC, 0]).reshape(P, H, W),
                "o": np.ascontiguousarray(output[c * BPC : (c + 1) * BPC, 0]).reshape(P, H, W),
                "op": np.ascontiguousarray(output_past[c * BPC : (c + 1) * BPC, 0]).reshape(P, H, W),
                "tg": np.ascontiguousarray(target[c * BPC : (c + 1) * BPC, 0]).reshape(P, H, W),
                "wmd": _bf16(md),
                "wid": _bf16(ident),
                "wdt": _bf16(ident * inv_dt_p[None, :]),
                "wsh": _bf16(wsh),
                "wsl": _bf16(wsl),
            }
        )
    return in_maps


def kernel(input, output, output_past, t, t_past, target):
    nc = get_nc()
    in_maps = build_in_maps(input, output, output_past, t, t_past, target)
    res = run_bass_kernel_spmd(nc, in_maps, list(range(N_CORES)))
    p_sum = 0.0
    m_sum = 0.0
    for r in res.results:
        p_sum += r["accp"].astype(np.float64).sum()
        m_sum += r["accm"].astype(np.float64).sum()
    return np.float32((p_sum + m_sum) / N_TOTAL)
